# revision 35
# baseline (speedup 1.0000x reference)
import os
import time
import numpy as np

# Constants matching the reference module (hardcoded; kernel.py must be self-contained)
N_PIX = 512
NV = 64
PIXSCALE = 0.05
FOV_HALF = 0.5 * (N_PIX - 1) * PIXSCALE  # 12.775
VEL0 = -400.0
DV = 12.5

N_CORES = 8
CH_PER_CORE = NV // N_CORES  # 8 channels per core
PAD = 3                      # 7x7 kernel -> reflect pad 3
NP_PAD = N_PIX + 2 * PAD     # 518
U_CHUNKS = 5                 # ceil(518/128) -> u padded to 640
XB = 4                       # x blocks of 128

_last_exec_time_ns = None


# ----------------------------------------------------------------------------
# Host histogram (trilinear splat) -- v1a: done on host, conv on device
# ----------------------------------------------------------------------------
def _host_cube(pos_img, vel_chan, flux):
    ra = np.ascontiguousarray(pos_img[..., 0].reshape(-1), dtype=np.float32)
    dec = np.ascontiguousarray(pos_img[..., 1].reshape(-1), dtype=np.float32)
    vel = np.ascontiguousarray(vel_chan.reshape(-1), dtype=np.float32)
    flx = np.ascontiguousarray(flux.reshape(-1), dtype=np.float32)

    gx = (ra + np.float32(FOV_HALF)) / np.float32(PIXSCALE)
    gy = (dec + np.float32(FOV_HALF)) / np.float32(PIXSCALE)
    gv = (vel - np.float32(VEL0)) / np.float32(DV)

    ix0f = np.floor(gx); iy0f = np.floor(gy); iv0f = np.floor(gv)
    fx = gx - ix0f; fy = gy - iy0f; fv = gv - iv0f
    ix0 = ix0f.astype(np.int32); iy0 = iy0f.astype(np.int32); iv0 = iv0f.astype(np.int32)

    mask = ((ix0 >= 0) & (ix0 < N_PIX - 1) &
            (iy0 >= 0) & (iy0 < N_PIX - 1) &
            (iv0 >= 0) & (iv0 < NV - 1))
    flx_m = np.where(mask, flx, np.float32(0.0))
    ix0 = np.clip(ix0, 0, N_PIX - 2)
    iy0 = np.clip(iy0, 0, N_PIX - 2)
    iv0 = np.clip(iv0, 0, NV - 2)

    wx0 = np.float32(1.0) - fx
    wy0 = np.float32(1.0) - fy
    wv0 = np.float32(1.0) - fv

    size = NV * N_PIX * N_PIX
    base = (iv0.astype(np.int64) * N_PIX + iy0) * N_PIX + ix0
    acc = np.zeros(size, dtype=np.float64)
    corners = [
        (0, 0, 0, wx0 * wy0 * wv0), (0, 1, 0, wx0 * fy * wv0),
        (0, 0, 1, fx * wy0 * wv0),  (0, 1, 1, fx * fy * wv0),
        (1, 0, 0, wx0 * wy0 * fv),  (1, 1, 0, wx0 * fy * fv),
        (1, 0, 1, fx * wy0 * fv),   (1, 1, 1, fx * fy * fv),
    ]
    for dv, dy, dx, w in corners:
        idx = base + (dv * N_PIX + dy) * N_PIX + dx
        acc += np.bincount(idx, weights=(flx_m * w).astype(np.float64), minlength=size)
    return acc.astype(np.float32).reshape(NV, N_PIX, N_PIX)


# ----------------------------------------------------------------------------
# Device conv kernel (built once per process)
# ----------------------------------------------------------------------------
_conv_nc = None


def _build_conv_nc():
    from concourse import bass, mybir

    nc = bass.Bass()
    bf16 = mybir.dt.bfloat16
    f32 = mybir.dt.float32

    IT_W = CH_PER_CORE * U_CHUNKS * NP_PAD
    BM_W = 7 * XB * 2 * 128
    ST_W = CH_PER_CORE * XB * N_PIX

    # inputs (per core): transposed padded slab + band matrices (pre-flattened)
    it_pad = nc.declare_dram_parameter("it_pad", [128, IT_W], bf16, isOutput=False)
    bmat = nc.declare_dram_parameter("bmat", [128, BM_W], bf16, isOutput=False)
    out_t = nc.declare_dram_parameter("out_t", [128, ST_W], f32, isOutput=True)

    it_tile = nc.alloc_sbuf_tensor("it_tile", [128, IT_W], bf16)
    b_tile = nc.alloc_sbuf_tensor("b_tile", [128, BM_W], bf16)
    stage = nc.alloc_sbuf_tensor("stage", [128, ST_W], f32)
    psum = [nc.alloc_psum_tensor(f"ps{i}", [128, N_PIX], f32) for i in range(8)]

    chains = [(ch, xb) for ch in range(CH_PER_CORE) for xb in range(XB)]
    n_chains = len(chains)

    with (nc.Block() as block,
          nc.semaphore("in_dma") as in_dma,
          nc.semaphore("mm_sem") as mm_sem,
          nc.semaphore("ev_sem") as ev_sem,
          nc.semaphore("out_dma") as out_dma):

        @block.sync
        def _(sync):
            sync.dma_start(out=it_tile[:], in_=it_pad[:]).then_inc(in_dma, 16)
            sync.dma_start(out=b_tile[:], in_=bmat[:]).then_inc(in_dma, 16)

        @block.tensor
        def _(tensor):
            tensor.wait_ge(in_dma, 32)
            for i, (ch, xb) in enumerate(chains):
                if i >= 8:
                    tensor.wait_ge(ev_sem, i - 7)
                ps = psum[i % 8]
                n_mm = 0
                for ky in range(7):
                    for s in range(2):
                        rhs_base = (ch * U_CHUNKS + xb + s) * NP_PAD
                        lhs_base = ((ky * XB + xb) * 2 + s) * 128
                        mm = tensor.matmul(
                            ps[:],
                            b_tile[:, lhs_base:lhs_base + 128],
                            it_tile[:, rhs_base + ky:rhs_base + ky + N_PIX],
                            start=(n_mm == 0),
                            stop=(n_mm == 13),
                        )
                        n_mm += 1
                mm.then_inc(mm_sem, 1)

        @block.vector
        def _(vector):
            for i, (ch, xb) in enumerate(chains):
                vector.wait_ge(mm_sem, i + 1)
                off = (ch * XB + xb) * N_PIX
                vector.tensor_copy(
                    out=stage[:, off:off + N_PIX], in_=psum[i % 8][:],
                ).then_inc(ev_sem, 1)

        @block.scalar
        def _(scalar):
            scalar.wait_ge(ev_sem, n_chains)
            scalar.dma_start(out=out_t[:], in_=stage[:]).then_inc(out_dma, 16)
            scalar.wait_ge(out_dma, 16)

    return nc


_runner_cache = {}


def _make_spmd_runner(nc, n_cores):
    """Build a reusable jitted SPMD executor for a Bass module (axon/PJRT path).

    Mirrors concourse.bass2jax.run_bass_via_pjrt but returns a callable we
    can invoke repeatedly for warm timing. Returns (fn, in_names, out_names,
    out_avals) where fn(concat_inputs, concat_zeros) -> list of out arrays.
    """
    import jax
    from jax.sharding import Mesh, PartitionSpec
    from jax.experimental.shard_map import shard_map
    from concourse import bass2jax, mybir

    bass2jax.install_neuronx_cc_hook()
    _bass_exec_p = bass2jax._bass_exec_p

    partition_name = nc.partition_id_tensor.name if nc.partition_id_tensor else None
    in_names, out_names, out_avals, zero_outs = [], [], [], []
    for alloc in nc.m.functions[0].allocations:
        if not isinstance(alloc, mybir.MemoryLocationSet):
            continue
        name = alloc.memorylocations[0].name
        if alloc.kind == "ExternalInput":
            if name != partition_name:
                in_names.append(name)
        elif alloc.kind == "ExternalOutput":
            shape = tuple(alloc.tensor_shape)
            dtype = mybir.dt.np(alloc.dtype)
            out_avals.append(jax.core.ShapedArray(shape, dtype))
            out_names.append(name)
            zero_outs.append(np.zeros(shape, dtype))
    n_params = len(in_names)
    n_outs = len(out_names)
    all_in_names = list(in_names) + list(out_names)
    if partition_name is not None:
        all_in_names.append(partition_name)

    donate = tuple(range(n_params, n_params + n_outs))

    def _body(*args):
        operands = list(args)
        if partition_name is not None:
            operands.append(bass2jax.partition_id_tensor())
        outs = _bass_exec_p.bind(
            *operands,
            out_avals=tuple(out_avals),
            in_names=tuple(all_in_names),
            out_names=tuple(out_names),
            lowering_input_output_aliases=(),
            sim_require_finite=True,
            sim_require_nnan=True,
            nc=nc,
        )
        return tuple(outs)

    devices = jax.devices()[:n_cores]
    mesh = Mesh(np.asarray(devices), ("core",))
    in_specs = (PartitionSpec("core"),) * (n_params + n_outs)
    out_specs = (PartitionSpec("core"),) * n_outs
    fn = jax.jit(
        shard_map(_body, mesh=mesh, in_specs=in_specs, out_specs=out_specs,
                  check_rep=False),
        donate_argnums=donate, keep_unused=True)
    sharding = jax.sharding.NamedSharding(mesh, PartitionSpec("core"))
    return fn, in_names, out_names, out_avals, zero_outs, sharding


def _run_spmd_timed(key, nc, in_maps, n_cores=N_CORES, n_timed=3):
    """Run the module on n_cores; return (results_per_core, best_exec_ns)."""
    import jax

    if key not in _runner_cache:
        _runner_cache[key] = _make_spmd_runner(nc, n_cores)
    fn, in_names, out_names, out_avals, zero_outs, sharding = _runner_cache[key]

    concat_in = [np.concatenate([np.asarray(in_maps[c][nm]) for c in range(n_cores)],
                                axis=0) for nm in in_names]
    dev_in = [jax.device_put(a, sharding) for a in concat_in]

    def zeros():
        return [jax.device_put(np.zeros((n_cores * z.shape[0], *z.shape[1:]),
                                        z.dtype), sharding) for z in zero_outs]

    # warm-up (includes compile on first call)
    out_arrs = fn(*dev_in, *zeros())
    out_arrs = [o.block_until_ready() for o in out_arrs]
    results = [
        {nm: np.asarray(out_arrs[i]).reshape(n_cores, *out_avals[i].shape)[c]
         for i, nm in enumerate(out_names)}
        for c in range(n_cores)
    ]

    best_ns = None
    if n_timed > 0:
        zsets = [zeros() for _ in range(n_timed)]
        for zs in zsets:
            for z in zs:
                z.block_until_ready()
        for zs in zsets:
            t0 = time.perf_counter()
            outs = fn(*dev_in, *zs)
            for o in outs:
                o.block_until_ready()
            dt = time.perf_counter() - t0
            ns = int(dt * 1e9)
            best_ns = ns if best_ns is None else min(best_ns, ns)
    return results, best_ns


def _conv_on_device(cube, kernel2d):
    """cube: (64, 512, 512) f32; returns conv output (64, 512, 512) f32."""
    global _conv_nc, _last_exec_time_ns
    from concourse import mybir

    bf = mybir.dt.np(mybir.dt.bfloat16)
    k2d = np.asarray(kernel2d, dtype=np.float32)

    # Band matrices B_ky[u, x] = K[ky, u - x], u - x in [0, 7)
    b_full = np.zeros((7, U_CHUNKS * 128, N_PIX), dtype=np.float32)
    ar = np.arange(N_PIX)
    for ky in range(7):
        for kx in range(7):
            b_full[ky, ar + kx, ar] = k2d[ky, kx]
    bmat = np.zeros((7, XB, 2, 128, 128), dtype=np.float32)
    for ky in range(7):
        for xb in range(XB):
            for s in range(2):
                u0 = 128 * (xb + s)
                bmat[ky, xb, s] = b_full[ky, u0:u0 + 128, 128 * xb:128 * xb + 128]
    # device layout: [p(=u within chunk), ky, xb, s, r]
    bmat = np.ascontiguousarray(bmat.transpose(3, 0, 1, 2, 4)).astype(bf).reshape(128, -1)

    in_maps = []
    for c in range(N_CORES):
        slab = cube[c * CH_PER_CORE:(c + 1) * CH_PER_CORE]
        itp = np.zeros((CH_PER_CORE, U_CHUNKS * 128, NP_PAD), dtype=np.float32)
        for ch in range(CH_PER_CORE):
            p = np.pad(slab[ch], PAD, mode="reflect")  # (518, 518) [y_pad, x_pad]
            itp[ch, :NP_PAD, :] = p.T                  # [u=x_pad, y_pad]
        itp = np.ascontiguousarray(
            itp.reshape(CH_PER_CORE, U_CHUNKS, 128, NP_PAD).transpose(2, 0, 1, 3)
        ).astype(bf).reshape(128, -1)
        in_maps.append({"it_pad": itp, "bmat": bmat})

    if _conv_nc is None:
        _conv_nc = _build_conv_nc()

    n_timed = int(os.environ.get("KERNEL_TIMED_RUNS", "3"))
    results, best_ns = _run_spmd_timed("conv", _conv_nc, in_maps, N_CORES, n_timed)
    _last_exec_time_ns = best_ns

    out = np.empty((NV, N_PIX, N_PIX), dtype=np.float32)
    for c in range(N_CORES):
        ot = results[c]["out_t"].reshape(128, CH_PER_CORE, XB, N_PIX)
        # [x_in_blk, ch, xb, y] -> [ch, y, xb, x_in_blk]
        out[c * CH_PER_CORE:(c + 1) * CH_PER_CORE] = (
            ot.transpose(1, 3, 2, 0).reshape(CH_PER_CORE, N_PIX, N_PIX))
    return out


# ----------------------------------------------------------------------------
# v1b: device-side histogram (matmul scatter)
# ----------------------------------------------------------------------------
CAP = 60          # tiles per (vl, xblk) group
NGROUPS = 9 * 8   # vl 0..8  x  xblk 0..7
NT = NGROUPS * CAP  # 4320 tiles per core
BATCH = 16        # tiles per DVE build batch (480 tiles/vl divisible by 16)

_scatter_nc = None


def _prep_emissions(pos_img, vel_chan, flux):
    """Compute emission field arrays per core. Returns list of dicts or None
    if a group overflows CAP (caller falls back to host cube)."""
    ra = np.ascontiguousarray(pos_img[..., 0].reshape(-1), dtype=np.float32)
    dec = np.ascontiguousarray(pos_img[..., 1].reshape(-1), dtype=np.float32)
    vel = np.ascontiguousarray(vel_chan.reshape(-1), dtype=np.float32)
    flx = np.ascontiguousarray(flux.reshape(-1), dtype=np.float32)

    gx = (ra + np.float32(FOV_HALF)) / np.float32(PIXSCALE)
    gy = (dec + np.float32(FOV_HALF)) / np.float32(PIXSCALE)
    gv = (vel - np.float32(VEL0)) / np.float32(DV)
    ix0 = np.floor(gx).astype(np.int32); fx = gx - np.floor(gx)
    iy0 = np.floor(gy).astype(np.int32); fy = gy - np.floor(gy)
    iv0 = np.floor(gv).astype(np.int32); fv = gv - np.floor(gv)
    mask = ((ix0 >= 0) & (ix0 < N_PIX - 1) & (iy0 >= 0) & (iy0 < N_PIX - 1) &
            (iv0 >= 0) & (iv0 < NV - 1))
    ix0, iy0, iv0 = ix0[mask], iy0[mask], iv0[mask]
    fx, fy, fv, flxm = fx[mask], fy[mask], fv[mask], flx[mask]

    ylo = (iy0 & 63).astype(np.float32); yhi = (iy0 >> 6).astype(np.float32)
    xlo = (ix0 & 63).astype(np.float32); xblk = (ix0 >> 6).astype(np.int32)
    ysplit = (iy0 & 63) == 63
    xsplit = (ix0 & 63) == 63

    f32 = np.float32
    one = f32(1.0)

    # Streams: (ylo, fy, yhi, xlo, fx, xblk, pidx). Splits move the second
    # tap to a new (yhi/xblk) with center offset -1 (ylo/xlo = -1).
    nys = ysplit.sum(); nxs = xsplit.sum()
    bm = ysplit & xsplit; nb = bm.sum()
    pidx = np.arange(len(fx), dtype=np.int64)

    def cat(*arrs):
        return np.concatenate(arrs)

    # For the hat to produce only the first tap on a split emission, clamp the
    # center so the second tap falls outside the valid range: for ysplit, the
    # hat at index 64 does not exist in a 64-wide row block, so no clamp needed
    # (the tap simply is not generated); same for xsplit within a 64 block.
    E_ylo = cat(ylo, -np.ones(nys, f32), ylo[xsplit], -np.ones(nb, f32))
    E_fy = cat(fy, fy[ysplit], fy[xsplit], fy[bm])
    E_yhi = cat(yhi, yhi[ysplit] + 1, yhi[xsplit], yhi[bm] + 1)
    E_xlo = cat(xlo, xlo[ysplit], -np.ones(nxs, f32), -np.ones(nb, f32))
    E_fx = cat(fx, fx[ysplit], fx[xsplit], fx[bm])
    E_xblk = cat(xblk, xblk[ysplit], xblk[xsplit] + 1, xblk[bm] + 1)
    E_pidx = cat(pidx, pidx[ysplit], pidx[xsplit], pidx[bm])

    w0 = (flxm * (one - fv)); w1 = (flxm * fv)
    ev = iv0[E_pidx]
    ew0 = w0[E_pidx]; ew1 = w1[E_pidx]
    vb = (ev & 7) == 7
    nvb = vb.sum()
    core1 = (ev >> 3).astype(np.int32)
    vl1 = ((ev & 7) + 1).astype(np.int32)
    w1_here = np.where(vb, f32(0.0), ew1)

    A_core = cat(core1, core1[vb] + 1)
    A_vl = cat(vl1, np.zeros(nvb, np.int32))
    A_w0 = cat(ew0, np.zeros(nvb, f32))
    A_w1 = cat(w1_here, ew1[vb])
    rep = lambda a: cat(a, a[vb])
    A_ylo, A_fy, A_yhi = rep(E_ylo), rep(E_fy), rep(E_yhi)
    A_xlo, A_fx, A_xblk = rep(E_xlo), rep(E_fx), rep(E_xblk)

    key = (A_core * 9 + A_vl) * 8 + A_xblk
    cnt = np.bincount(key, minlength=8 * NGROUPS)
    if cnt.max() > CAP * 128:
        return None

    order = np.argsort(key, kind="stable")
    key_s = key[order]
    starts = np.zeros(8 * NGROUPS + 1, np.int64)
    np.cumsum(cnt, out=starts[1:])
    # slot within the global padded field array (all cores concatenated)
    grp_base = np.arange(8 * NGROUPS, dtype=np.int64) * (CAP * 128)
    seq = np.arange(len(key_s), dtype=np.int64) - starts[key_s]
    slots = grp_base[key_s] + seq

    bf = np.dtype(np.float32)  # staging in f32; cast at the end
    total = 8 * NT * 128
    fields = {
        "ylo": A_ylo, "fyp1": A_fy + one, "yhi": A_yhi,
        "xlo": A_xlo, "fxp1": A_fx + one, "w0": A_w0, "w1": A_w1,
    }
    out = []
    flat = {}
    for nm, vals in fields.items():
        a = np.zeros(total, bf)
        a[slots] = vals[order]
        flat[nm] = a
    import ml_dtypes
    bf16 = ml_dtypes.bfloat16
    for c in range(N_CORES):
        m = {}
        for nm, a in flat.items():
            sub = a[c * NT * 128:(c + 1) * NT * 128]
            # slot s -> tile s//128, partition s%128 ; device layout [128, NT]
            m[nm] = np.ascontiguousarray(
                sub.reshape(NT, 128).T).astype(bf16)
        out.append(m)
    return out


def _build_scatter_nc(reps=1):
    from concourse import bass, mybir

    nc = bass.Bass()
    bf16 = mybir.dt.bfloat16
    f32 = mybir.dt.float32
    B = BATCH
    AluOp = mybir.AluOpType

    FIELD_NAMES = ["ylo", "fyp1", "yhi", "xlo", "fxp1", "w0", "w1"]
    f_in = {nm: nc.declare_dram_parameter(nm, [128, NT], bf16, isOutput=False)
            for nm in FIELD_NAMES}
    consts = nc.declare_dram_parameter("consts", [128, 72], bf16, isOutput=False)
    # out slab: [128=(sub2, ylo64), (pair4, yhi8, x512)]
    out_slab = nc.declare_dram_parameter("out_slab", [128, 4 * 8 * N_PIX], bf16,
                                         isOutput=True)

    f_sb = {nm: nc.alloc_sbuf_tensor(f"sb_{nm}", [128, NT], bf16)
            for nm in FIELD_NAMES}
    c_sb = nc.alloc_sbuf_tensor("sb_consts", [128, 72], bf16)
    slab = nc.alloc_sbuf_tensor("slab", [128, 4, 8, N_PIX], bf16)

    lhs_buf = [nc.alloc_sbuf_tensor(f"lhs{i}", [128, B, 128], bf16) for i in (0, 1)]
    rhs_buf = [nc.alloc_sbuf_tensor(f"rhs{i}", [128, B, 512], bf16) for i in (0, 1)]
    scr = [nc.alloc_sbuf_tensor(f"scr{i}", [128, B, 64], bf16) for i in range(6)]
    scr_s = [nc.alloc_sbuf_tensor(f"scrs{i}", [128, B], bf16) for i in range(2)]
    a8_sb = nc.alloc_sbuf_tensor("a8b", [128, B, 8], bf16)

    psum = [nc.alloc_psum_tensor(f"ps{i}", [128, 512], f32) for i in range(8)]

    NBATCH = (NT // B) * reps
    BPV = (NT // B) // 9

    def repi(ap, n):
        # [128, B] -> [128, B, n] (repeat each value n times, inner)
        return ap.unsqueeze(-1).broadcast_to((128, B, n))

    def repo(ap, n_outer, w):
        # [128, w] -> [128, n_outer, w] (tile w-vector n_outer times)
        return ap.unsqueeze(1).broadcast_to((128, n_outer, w))

    with (nc.Block() as block,
          nc.semaphore("f_dma") as f_dma,
          nc.semaphore("bld") as bld,
          nc.semaphore("used") as used,
          nc.semaphore("ev") as ev,
          nc.semaphore("out_dma") as out_dma):

        @block.sync
        def _(sync):
            for nm in FIELD_NAMES:
                sync.dma_start(out=f_sb[nm][:], in_=f_in[nm][:]).then_inc(f_dma, 16)
            sync.dma_start(out=c_sb[:], in_=consts[:]).then_inc(f_dma, 16)
            sync.wait_ge(ev, 9 * reps)
            sync.dma_start(out=out_slab[:],
                           in_=slab[:].rearrange("p a b c -> p (a b c)")
                           ).then_inc(out_dma, 16)
            sync.wait_ge(out_dma, 16)

        @block.vector
        def _(vector):
            vector.wait_ge(f_dma, 8 * 16)
            io64 = c_sb[:, 0:64]
            io8 = c_sb[:, 64:72]
            for b in range(NBATCH):
                vl = b // BPV
                if b >= 2:
                    vector.wait_ge(used, b - 1)
                lhs = lhs_buf[b % 2][:]
                rhs = rhs_buf[b % 2][:]
                t0 = (b % (NT // B)) * B
                fld = {nm: f_sb[nm][:, t0:t0 + B] for nm in FIELD_NAMES}

                d, dx = scr[0][:], scr[1][:]
                p, q = scr[2][:], scr[3][:]
                px, qx = scr[4][:], scr[5][:]
                fm1, fm2 = scr_s[0][:], scr_s[1][:]
                # G1: independent ops
                vector.tensor_tensor(out=d, in0=repo(io64, B, 64),
                                     in1=repi(fld["ylo"], 64), op=AluOp.subtract)
                vector.tensor_tensor(out=dx, in0=repo(io64, B, 64),
                                     in1=repi(fld["xlo"], 64), op=AluOp.subtract)
                vector.tensor_scalar_add(fm1, fld["fyp1"], -2.0)
                vector.tensor_scalar_add(fm2, fld["fxp1"], -2.0)
                vector.tensor_tensor(out=a8_sb[:], in0=repo(io8, B, 8),
                                     in1=repi(fld["yhi"], 8), op=AluOp.is_equal)
                vector.drain()
                # G2: reads d, dx, fm1, fm2
                vector.tensor_tensor(out=p, in0=repi(fld["fyp1"], 64), in1=d,
                                     op=AluOp.subtract)
                vector.tensor_tensor(out=q, in0=d, in1=repi(fm1, 64),
                                     op=AluOp.subtract)
                vector.tensor_tensor(out=px, in0=repi(fld["fxp1"], 64), in1=dx,
                                     op=AluOp.subtract)
                vector.tensor_tensor(out=qx, in0=dx, in1=repi(fm2, 64),
                                     op=AluOp.subtract)
                vector.drain()
                # G3: mins (reuse d, dx)
                hm, hmx = d, dx
                vector.tensor_tensor(out=hm, in0=p, in1=q, op=AluOp.min)
                vector.tensor_tensor(out=hmx, in0=px, in1=qx, op=AluOp.min)
                vector.drain()
                # G4: relu (reuse p, px)
                hy, hx = p, px
                vector.tensor_scalar_max(hy, hm, 0.0)
                vector.tensor_scalar_max(hx, hmx, 0.0)
                vector.drain()
                # G5: operand outputs
                vector.tensor_tensor(out=lhs[:, :, 0:64], in0=hy,
                                     in1=repi(fld["w0"], 64), op=AluOp.mult)
                vector.tensor_tensor(out=lhs[:, :, 64:128], in0=hy,
                                     in1=repi(fld["w1"], 64), op=AluOp.mult)
                rhs4 = rhs.rearrange("p b (a x) -> p b a x", a=8)
                mm_in0 = a8_sb[:].unsqueeze(-1).broadcast_to((128, B, 8, 64))
                mm_in1 = hx.unsqueeze(2).broadcast_to((128, B, 8, 64))
                vector.tensor_tensor(out=rhs4, in0=mm_in0, in1=mm_in1,
                                     op=AluOp.mult)
                vector.drain().then_inc(bld, 1)

                # --- evictions at vl boundaries (after builds of this batch)
                if b % BPV == BPV - 1:
                    vector.wait_ge(used, (vl + 1) * BPV)
                    for xblk in range(8):
                        for dv in range(2):
                            ch = (vl % 9) - 1 + dv
                            if ch < 0 or ch >= 8:
                                continue
                            pair, sub = divmod(ch, 2)
                            src = psum[xblk][dv * 64:(dv + 1) * 64, :]\
                                .rearrange("p (a x) -> p a x", a=8)
                            dst = slab[sub * 64:(sub + 1) * 64, pair, :,
                                       xblk * 64:(xblk + 1) * 64]
                            if dv == 1:
                                vector.tensor_copy(out=dst, in_=src)
                            else:
                                vector.tensor_add(out=dst, in0=dst, in1=src)
                    vector.drain().then_inc(ev, 1)

        @block.tensor
        def _(tensor):
            for b in range(NBATCH):
                vl = b // BPV
                if b % BPV == 0 and vl > 0:
                    tensor.wait_ge(ev, vl)
                tensor.wait_ge(bld, b + 1)
                lhs = lhs_buf[b % 2]
                rhs = rhs_buf[b % 2]
                last = None
                for j in range(B):
                    t = (b % (NT // B)) * B + j
                    grp = t // CAP           # global group id = vl*8 + xblk
                    xblk = grp % 8
                    tin = t % CAP
                    last = tensor.matmul(
                        psum[xblk][:],
                        lhs[:, j],
                        rhs[:, j],
                        start=(tin == 0),
                        stop=(tin == CAP - 1),
                    )
                last.then_inc(used, 1)

    return nc


def _scatter_on_device(em_maps):
    """Run the scatter NEFF; returns full cube (64, 512, 512) f32."""
    global _scatter_nc
    import ml_dtypes
    bf16 = ml_dtypes.bfloat16

    co = np.zeros((128, 72), np.float32)
    co[:, 0:64] = np.arange(64)[None, :]
    co[:, 64:72] = np.arange(8)[None, :]
    co = co.astype(bf16)
    in_maps = [{**em_maps[c], "consts": co} for c in range(N_CORES)]

    if _scatter_nc is None:
        _scatter_nc = _build_scatter_nc()

    n_timed = int(os.environ.get("KERNEL_TIMED_RUNS", "3"))
    results, best_ns = _run_spmd_timed("scatter", _scatter_nc, in_maps, N_CORES,
                                       n_timed)
    cube = np.empty((NV, N_PIX, N_PIX), dtype=np.float32)
    for c in range(N_CORES):
        sl = results[c]["out_slab"].astype(np.float32)
        sl = sl.reshape(2, 64, 4, 8, N_PIX)     # [sub, ylo, pair, yhi, x]
        # ch = pair*2 + sub ; y = yhi*64 + ylo
        cube[c * CH_PER_CORE:(c + 1) * CH_PER_CORE] = (
            sl.transpose(2, 0, 3, 1, 4)         # [pair, sub, yhi, ylo, x]
              .reshape(CH_PER_CORE, N_PIX, N_PIX))
    return cube, best_ns


def kernel(pos_img, vel_chan, flux, kernel2d):
    global _last_exec_time_ns
    scat_ns = None
    cube = None
    if os.environ.get("KERNEL_HOST_CUBE", "0") != "1":
        try:
            em = _prep_emissions(pos_img, vel_chan, flux)
            if em is not None:
                cube, scat_ns = _scatter_on_device(em)
        except Exception as e:
            import traceback
            traceback.print_exc()
            cube = None
    if cube is None:
        cube = _host_cube(pos_img, vel_chan, flux)
    out = _conv_on_device(cube, kernel2d)
    if scat_ns is not None and _last_exec_time_ns is not None:
        _last_exec_time_ns = _last_exec_time_ns + scat_ns
    return out


# revision 36
# speedup vs baseline: 2.1644x; 2.1644x over previous
import os
import time
import numpy as np

# Constants matching the reference module (hardcoded; kernel.py must be self-contained)
N_PIX = 512
NV = 64
PIXSCALE = 0.05
FOV_HALF = 0.5 * (N_PIX - 1) * PIXSCALE  # 12.775
VEL0 = -400.0
DV = 12.5

N_CORES = 8
CH_PER_CORE = NV // N_CORES  # 8 channels per core
PAD = 3                      # 7x7 kernel -> reflect pad 3
NP_PAD = N_PIX + 2 * PAD     # 518
U_CHUNKS = 5                 # ceil(518/128) -> u padded to 640
XB = 4                       # x blocks of 128

_last_exec_time_ns = None


# ----------------------------------------------------------------------------
# Host histogram (trilinear splat) -- v1a: done on host, conv on device
# ----------------------------------------------------------------------------
def _host_cube(pos_img, vel_chan, flux):
    ra = np.ascontiguousarray(pos_img[..., 0].reshape(-1), dtype=np.float32)
    dec = np.ascontiguousarray(pos_img[..., 1].reshape(-1), dtype=np.float32)
    vel = np.ascontiguousarray(vel_chan.reshape(-1), dtype=np.float32)
    flx = np.ascontiguousarray(flux.reshape(-1), dtype=np.float32)

    gx = (ra + np.float32(FOV_HALF)) / np.float32(PIXSCALE)
    gy = (dec + np.float32(FOV_HALF)) / np.float32(PIXSCALE)
    gv = (vel - np.float32(VEL0)) / np.float32(DV)

    ix0f = np.floor(gx); iy0f = np.floor(gy); iv0f = np.floor(gv)
    fx = gx - ix0f; fy = gy - iy0f; fv = gv - iv0f
    ix0 = ix0f.astype(np.int32); iy0 = iy0f.astype(np.int32); iv0 = iv0f.astype(np.int32)

    mask = ((ix0 >= 0) & (ix0 < N_PIX - 1) &
            (iy0 >= 0) & (iy0 < N_PIX - 1) &
            (iv0 >= 0) & (iv0 < NV - 1))
    flx_m = np.where(mask, flx, np.float32(0.0))
    ix0 = np.clip(ix0, 0, N_PIX - 2)
    iy0 = np.clip(iy0, 0, N_PIX - 2)
    iv0 = np.clip(iv0, 0, NV - 2)

    wx0 = np.float32(1.0) - fx
    wy0 = np.float32(1.0) - fy
    wv0 = np.float32(1.0) - fv

    size = NV * N_PIX * N_PIX
    base = (iv0.astype(np.int64) * N_PIX + iy0) * N_PIX + ix0
    acc = np.zeros(size, dtype=np.float64)
    corners = [
        (0, 0, 0, wx0 * wy0 * wv0), (0, 1, 0, wx0 * fy * wv0),
        (0, 0, 1, fx * wy0 * wv0),  (0, 1, 1, fx * fy * wv0),
        (1, 0, 0, wx0 * wy0 * fv),  (1, 1, 0, wx0 * fy * fv),
        (1, 0, 1, fx * wy0 * fv),   (1, 1, 1, fx * fy * fv),
    ]
    for dv, dy, dx, w in corners:
        idx = base + (dv * N_PIX + dy) * N_PIX + dx
        acc += np.bincount(idx, weights=(flx_m * w).astype(np.float64), minlength=size)
    return acc.astype(np.float32).reshape(NV, N_PIX, N_PIX)


# ----------------------------------------------------------------------------
# Device conv kernel (built once per process)
# ----------------------------------------------------------------------------
_conv_nc = None


def _build_conv_nc():
    from concourse import bass, mybir

    nc = bass.Bass()
    bf16 = mybir.dt.bfloat16
    f32 = mybir.dt.float32

    IT_W = CH_PER_CORE * U_CHUNKS * NP_PAD
    BM_W = 7 * XB * 2 * 128
    ST_W = CH_PER_CORE * XB * N_PIX

    # inputs (per core): transposed padded slab + band matrices (pre-flattened)
    it_pad = nc.declare_dram_parameter("it_pad", [128, IT_W], bf16, isOutput=False)
    bmat = nc.declare_dram_parameter("bmat", [128, BM_W], bf16, isOutput=False)
    out_t = nc.declare_dram_parameter("out_t", [128, ST_W], f32, isOutput=True)

    it_tile = nc.alloc_sbuf_tensor("it_tile", [128, IT_W], bf16)
    b_tile = nc.alloc_sbuf_tensor("b_tile", [128, BM_W], bf16)
    stage = nc.alloc_sbuf_tensor("stage", [128, ST_W], f32)
    psum = [nc.alloc_psum_tensor(f"ps{i}", [128, N_PIX], f32) for i in range(8)]

    chains = [(ch, xb) for ch in range(CH_PER_CORE) for xb in range(XB)]
    n_chains = len(chains)

    with (nc.Block() as block,
          nc.semaphore("in_dma") as in_dma,
          nc.semaphore("mm_sem") as mm_sem,
          nc.semaphore("ev_sem") as ev_sem,
          nc.semaphore("out_dma") as out_dma):

        @block.sync
        def _(sync):
            sync.dma_start(out=it_tile[:], in_=it_pad[:]).then_inc(in_dma, 16)
            sync.dma_start(out=b_tile[:], in_=bmat[:]).then_inc(in_dma, 16)

        @block.tensor
        def _(tensor):
            tensor.wait_ge(in_dma, 32)
            for i, (ch, xb) in enumerate(chains):
                if i >= 8:
                    tensor.wait_ge(ev_sem, i - 7)
                ps = psum[i % 8]
                n_mm = 0
                for ky in range(7):
                    for s in range(2):
                        rhs_base = (ch * U_CHUNKS + xb + s) * NP_PAD
                        lhs_base = ((ky * XB + xb) * 2 + s) * 128
                        mm = tensor.matmul(
                            ps[:],
                            b_tile[:, lhs_base:lhs_base + 128],
                            it_tile[:, rhs_base + ky:rhs_base + ky + N_PIX],
                            start=(n_mm == 0),
                            stop=(n_mm == 13),
                        )
                        n_mm += 1
                mm.then_inc(mm_sem, 1)

        @block.vector
        def _(vector):
            for i, (ch, xb) in enumerate(chains):
                vector.wait_ge(mm_sem, i + 1)
                off = (ch * XB + xb) * N_PIX
                vector.tensor_copy(
                    out=stage[:, off:off + N_PIX], in_=psum[i % 8][:],
                ).then_inc(ev_sem, 1)

        @block.scalar
        def _(scalar):
            scalar.wait_ge(ev_sem, n_chains)
            scalar.dma_start(out=out_t[:], in_=stage[:]).then_inc(out_dma, 16)
            scalar.wait_ge(out_dma, 16)

    return nc


_runner_cache = {}


def _make_spmd_runner(nc, n_cores):
    """Build a reusable jitted SPMD executor for a Bass module (axon/PJRT path).

    Mirrors concourse.bass2jax.run_bass_via_pjrt but returns a callable we
    can invoke repeatedly for warm timing. Returns (fn, in_names, out_names,
    out_avals) where fn(concat_inputs, concat_zeros) -> list of out arrays.
    """
    import jax
    from jax.sharding import Mesh, PartitionSpec
    from jax.experimental.shard_map import shard_map
    from concourse import bass2jax, mybir

    bass2jax.install_neuronx_cc_hook()
    _bass_exec_p = bass2jax._bass_exec_p

    partition_name = nc.partition_id_tensor.name if nc.partition_id_tensor else None
    in_names, out_names, out_avals, zero_outs = [], [], [], []
    for alloc in nc.m.functions[0].allocations:
        if not isinstance(alloc, mybir.MemoryLocationSet):
            continue
        name = alloc.memorylocations[0].name
        if alloc.kind == "ExternalInput":
            if name != partition_name:
                in_names.append(name)
        elif alloc.kind == "ExternalOutput":
            shape = tuple(alloc.tensor_shape)
            dtype = mybir.dt.np(alloc.dtype)
            out_avals.append(jax.core.ShapedArray(shape, dtype))
            out_names.append(name)
            zero_outs.append(np.zeros(shape, dtype))
    n_params = len(in_names)
    n_outs = len(out_names)
    all_in_names = list(in_names) + list(out_names)
    if partition_name is not None:
        all_in_names.append(partition_name)

    donate = tuple(range(n_params, n_params + n_outs))

    def _body(*args):
        operands = list(args)
        if partition_name is not None:
            operands.append(bass2jax.partition_id_tensor())
        outs = _bass_exec_p.bind(
            *operands,
            out_avals=tuple(out_avals),
            in_names=tuple(all_in_names),
            out_names=tuple(out_names),
            lowering_input_output_aliases=(),
            sim_require_finite=True,
            sim_require_nnan=True,
            nc=nc,
        )
        return tuple(outs)

    devices = jax.devices()[:n_cores]
    mesh = Mesh(np.asarray(devices), ("core",))
    in_specs = (PartitionSpec("core"),) * (n_params + n_outs)
    out_specs = (PartitionSpec("core"),) * n_outs
    fn = jax.jit(
        shard_map(_body, mesh=mesh, in_specs=in_specs, out_specs=out_specs,
                  check_rep=False),
        donate_argnums=donate, keep_unused=True)
    sharding = jax.sharding.NamedSharding(mesh, PartitionSpec("core"))
    return fn, in_names, out_names, out_avals, zero_outs, sharding


def _run_spmd_timed(key, nc, in_maps, n_cores=N_CORES, n_timed=3):
    """Run the module on n_cores; return (results_per_core, best_exec_ns)."""
    import jax

    if key not in _runner_cache:
        _runner_cache[key] = _make_spmd_runner(nc, n_cores)
    fn, in_names, out_names, out_avals, zero_outs, sharding = _runner_cache[key]

    concat_in = [np.concatenate([np.asarray(in_maps[c][nm]) for c in range(n_cores)],
                                axis=0) for nm in in_names]
    dev_in = [jax.device_put(a, sharding) for a in concat_in]

    def zeros():
        return [jax.device_put(np.zeros((n_cores * z.shape[0], *z.shape[1:]),
                                        z.dtype), sharding) for z in zero_outs]

    # warm-up (includes compile on first call)
    out_arrs = fn(*dev_in, *zeros())
    out_arrs = [o.block_until_ready() for o in out_arrs]
    results = [
        {nm: np.asarray(out_arrs[i]).reshape(n_cores, *out_avals[i].shape)[c]
         for i, nm in enumerate(out_names)}
        for c in range(n_cores)
    ]

    best_ns = None
    if n_timed > 0:
        zsets = [zeros() for _ in range(n_timed)]
        for zs in zsets:
            for z in zs:
                z.block_until_ready()
        for zs in zsets:
            t0 = time.perf_counter()
            outs = fn(*dev_in, *zs)
            for o in outs:
                o.block_until_ready()
            dt = time.perf_counter() - t0
            ns = int(dt * 1e9)
            best_ns = ns if best_ns is None else min(best_ns, ns)
    return results, best_ns


def _conv_on_device(cube, kernel2d):
    """cube: (64, 512, 512) f32; returns conv output (64, 512, 512) f32."""
    global _conv_nc, _last_exec_time_ns
    from concourse import mybir

    bf = mybir.dt.np(mybir.dt.bfloat16)
    k2d = np.asarray(kernel2d, dtype=np.float32)

    # Band matrices B_ky[u, x] = K[ky, u - x], u - x in [0, 7)
    b_full = np.zeros((7, U_CHUNKS * 128, N_PIX), dtype=np.float32)
    ar = np.arange(N_PIX)
    for ky in range(7):
        for kx in range(7):
            b_full[ky, ar + kx, ar] = k2d[ky, kx]
    bmat = np.zeros((7, XB, 2, 128, 128), dtype=np.float32)
    for ky in range(7):
        for xb in range(XB):
            for s in range(2):
                u0 = 128 * (xb + s)
                bmat[ky, xb, s] = b_full[ky, u0:u0 + 128, 128 * xb:128 * xb + 128]
    # device layout: [p(=u within chunk), ky, xb, s, r]
    bmat = np.ascontiguousarray(bmat.transpose(3, 0, 1, 2, 4)).astype(bf).reshape(128, -1)

    in_maps = []
    for c in range(N_CORES):
        slab = cube[c * CH_PER_CORE:(c + 1) * CH_PER_CORE]
        itp = np.zeros((CH_PER_CORE, U_CHUNKS * 128, NP_PAD), dtype=np.float32)
        for ch in range(CH_PER_CORE):
            p = np.pad(slab[ch], PAD, mode="reflect")  # (518, 518) [y_pad, x_pad]
            itp[ch, :NP_PAD, :] = p.T                  # [u=x_pad, y_pad]
        itp = np.ascontiguousarray(
            itp.reshape(CH_PER_CORE, U_CHUNKS, 128, NP_PAD).transpose(2, 0, 1, 3)
        ).astype(bf).reshape(128, -1)
        in_maps.append({"it_pad": itp, "bmat": bmat})

    if _conv_nc is None:
        _conv_nc = _build_conv_nc()

    n_timed = int(os.environ.get("KERNEL_TIMED_RUNS", "3"))
    results, best_ns = _run_spmd_timed("conv", _conv_nc, in_maps, N_CORES, n_timed)
    _last_exec_time_ns = best_ns

    out = np.empty((NV, N_PIX, N_PIX), dtype=np.float32)
    for c in range(N_CORES):
        ot = results[c]["out_t"].reshape(128, CH_PER_CORE, XB, N_PIX)
        # [x_in_blk, ch, xb, y] -> [ch, y, xb, x_in_blk]
        out[c * CH_PER_CORE:(c + 1) * CH_PER_CORE] = (
            ot.transpose(1, 3, 2, 0).reshape(CH_PER_CORE, N_PIX, N_PIX))
    return out


# ----------------------------------------------------------------------------
# v1b: device-side histogram (matmul scatter)
# ----------------------------------------------------------------------------
CAP = 58          # tiles per (vl, xblk) group (observed max 57.2; +7 sigma margin)
NGROUPS = 9 * 8   # vl 0..8  x  xblk 0..7
NT = NGROUPS * CAP  # 4320 tiles per core
BATCH = 16        # tiles per DVE build batch (480 tiles/vl divisible by 16)

_scatter_nc = None


def _prep_emissions(pos_img, vel_chan, flux):
    """Compute emission field arrays per core. Returns list of dicts or None
    if a group overflows CAP (caller falls back to host cube)."""
    ra = np.ascontiguousarray(pos_img[..., 0].reshape(-1), dtype=np.float32)
    dec = np.ascontiguousarray(pos_img[..., 1].reshape(-1), dtype=np.float32)
    vel = np.ascontiguousarray(vel_chan.reshape(-1), dtype=np.float32)
    flx = np.ascontiguousarray(flux.reshape(-1), dtype=np.float32)

    gx = (ra + np.float32(FOV_HALF)) / np.float32(PIXSCALE)
    gy = (dec + np.float32(FOV_HALF)) / np.float32(PIXSCALE)
    gv = (vel - np.float32(VEL0)) / np.float32(DV)
    ix0 = np.floor(gx).astype(np.int32); fx = gx - np.floor(gx)
    iy0 = np.floor(gy).astype(np.int32); fy = gy - np.floor(gy)
    iv0 = np.floor(gv).astype(np.int32); fv = gv - np.floor(gv)
    mask = ((ix0 >= 0) & (ix0 < N_PIX - 1) & (iy0 >= 0) & (iy0 < N_PIX - 1) &
            (iv0 >= 0) & (iv0 < NV - 1))
    ix0, iy0, iv0 = ix0[mask], iy0[mask], iv0[mask]
    fx, fy, fv, flxm = fx[mask], fy[mask], fv[mask], flx[mask]

    ylo = (iy0 & 63).astype(np.float32); yhi = (iy0 >> 6).astype(np.float32)
    xlo = (ix0 & 63).astype(np.float32); xblk = (ix0 >> 6).astype(np.int32)
    ysplit = (iy0 & 63) == 63
    xsplit = (ix0 & 63) == 63

    f32 = np.float32
    one = f32(1.0)

    # Streams: (ylo, fy, yhi, xlo, fx, xblk, pidx). Splits move the second
    # tap to a new (yhi/xblk) with center offset -1 (ylo/xlo = -1).
    nys = ysplit.sum(); nxs = xsplit.sum()
    bm = ysplit & xsplit; nb = bm.sum()
    pidx = np.arange(len(fx), dtype=np.int64)

    def cat(*arrs):
        return np.concatenate(arrs)

    # For the hat to produce only the first tap on a split emission, clamp the
    # center so the second tap falls outside the valid range: for ysplit, the
    # hat at index 64 does not exist in a 64-wide row block, so no clamp needed
    # (the tap simply is not generated); same for xsplit within a 64 block.
    E_ylo = cat(ylo, -np.ones(nys, f32), ylo[xsplit], -np.ones(nb, f32))
    E_fy = cat(fy, fy[ysplit], fy[xsplit], fy[bm])
    E_yhi = cat(yhi, yhi[ysplit] + 1, yhi[xsplit], yhi[bm] + 1)
    E_xlo = cat(xlo, xlo[ysplit], -np.ones(nxs, f32), -np.ones(nb, f32))
    E_fx = cat(fx, fx[ysplit], fx[xsplit], fx[bm])
    E_xblk = cat(xblk, xblk[ysplit], xblk[xsplit] + 1, xblk[bm] + 1)
    E_pidx = cat(pidx, pidx[ysplit], pidx[xsplit], pidx[bm])

    w0 = (flxm * (one - fv)); w1 = (flxm * fv)
    ev = iv0[E_pidx]
    ew0 = w0[E_pidx]; ew1 = w1[E_pidx]
    vb = (ev & 7) == 7
    nvb = vb.sum()
    core1 = (ev >> 3).astype(np.int32)
    vl1 = ((ev & 7) + 1).astype(np.int32)
    w1_here = np.where(vb, f32(0.0), ew1)

    A_core = cat(core1, core1[vb] + 1)
    A_vl = cat(vl1, np.zeros(nvb, np.int32))
    A_w0 = cat(ew0, np.zeros(nvb, f32))
    A_w1 = cat(w1_here, ew1[vb])
    rep = lambda a: cat(a, a[vb])
    A_ylo, A_fy, A_yhi = rep(E_ylo), rep(E_fy), rep(E_yhi)
    A_xlo, A_fx, A_xblk = rep(E_xlo), rep(E_fx), rep(E_xblk)

    key = (A_core * 9 + A_vl) * 8 + A_xblk
    cnt = np.bincount(key, minlength=8 * NGROUPS)
    if cnt.max() > CAP * 128:
        return None

    order = np.argsort(key, kind="stable")
    key_s = key[order]
    starts = np.zeros(8 * NGROUPS + 1, np.int64)
    np.cumsum(cnt, out=starts[1:])
    # slot within the global padded field array (all cores concatenated)
    grp_base = np.arange(8 * NGROUPS, dtype=np.int64) * (CAP * 128)
    seq = np.arange(len(key_s), dtype=np.int64) - starts[key_s]
    slots = grp_base[key_s] + seq

    bf = np.dtype(np.float32)  # staging in f32; cast at the end
    total = 8 * NT * 128
    fields = {
        "ylo": A_ylo, "fyp1": A_fy + one, "yhi": A_yhi,
        "xlo": A_xlo, "fxp1": A_fx + one, "w0": A_w0, "w1": A_w1,
    }
    out = []
    flat = {}
    for nm, vals in fields.items():
        a = np.zeros(total, bf)
        a[slots] = vals[order]
        flat[nm] = a
    import ml_dtypes
    bf16 = ml_dtypes.bfloat16
    for c in range(N_CORES):
        m = {}
        for nm, a in flat.items():
            sub = a[c * NT * 128:(c + 1) * NT * 128]
            # slot s -> tile s//128, partition s%128 ; device layout [128, NT]
            m[nm] = np.ascontiguousarray(
                sub.reshape(NT, 128).T).astype(bf16)
        out.append(m)
    return out


def _build_scatter_nc(reps=1):
    from concourse import bass, mybir

    nc = bass.Bass()
    bf16 = mybir.dt.bfloat16
    f32 = mybir.dt.float32
    B = BATCH
    AluOp = mybir.AluOpType

    FIELD_NAMES = ["ylo", "fyp1", "yhi", "xlo", "fxp1", "w0", "w1"]
    f_in = {nm: nc.declare_dram_parameter(nm, [128, NT], bf16, isOutput=False)
            for nm in FIELD_NAMES}
    consts = nc.declare_dram_parameter("consts", [128, 72], bf16, isOutput=False)
    # out slab: [128=(sub2, ylo64), (pair4, yhi8, x512)]
    out_slab = nc.declare_dram_parameter("out_slab", [128, 4 * 8 * N_PIX], bf16,
                                         isOutput=True)

    f_sb = {nm: nc.alloc_sbuf_tensor(f"sb_{nm}", [128, NT], bf16)
            for nm in FIELD_NAMES}
    c_sb = nc.alloc_sbuf_tensor("sb_consts", [128, 72], bf16)
    slab = nc.alloc_sbuf_tensor("slab", [128, 4, 8, N_PIX], bf16)

    lhs_buf = [nc.alloc_sbuf_tensor(f"lhs{i}", [128, B, 128], bf16) for i in (0, 1)]
    rhs_buf = [nc.alloc_sbuf_tensor(f"rhs{i}", [128, B, 512], bf16) for i in (0, 1)]
    scr = [nc.alloc_sbuf_tensor(f"scr{i}", [128, B, 64], bf16) for i in range(6)]
    scr_s = [nc.alloc_sbuf_tensor(f"scrs{i}", [128, B], bf16) for i in range(2)]
    a8_sb = nc.alloc_sbuf_tensor("a8b", [128, B, 8], bf16)

    psum = [nc.alloc_psum_tensor(f"ps{i}", [128, 512], f32) for i in range(8)]

    NBATCH = (NT // B) * reps
    BPV = (NT // B) // 9

    def repi(ap, n):
        # [128, B] -> [128, B, n] (repeat each value n times, inner)
        return ap.unsqueeze(-1).broadcast_to((128, B, n))

    def repo(ap, n_outer, w):
        # [128, w] -> [128, n_outer, w] (tile w-vector n_outer times)
        return ap.unsqueeze(1).broadcast_to((128, n_outer, w))

    with (nc.Block() as block,
          nc.semaphore("f_dma") as f_dma,
          nc.semaphore("bld") as bld,
          nc.semaphore("used") as used,
          nc.semaphore("ev") as ev,
          nc.semaphore("out_dma") as out_dma):

        @block.sync
        def _(sync):
            for nm in FIELD_NAMES:
                sync.dma_start(out=f_sb[nm][:], in_=f_in[nm][:]).then_inc(f_dma, 16)
            sync.dma_start(out=c_sb[:], in_=consts[:]).then_inc(f_dma, 16)
            sync.wait_ge(ev, 9 * reps)
            sync.dma_start(out=out_slab[:],
                           in_=slab[:].rearrange("p a b c -> p (a b c)")
                           ).then_inc(out_dma, 16)
            sync.wait_ge(out_dma, 16)

        @block.vector
        def _(vector):
            vector.wait_ge(f_dma, 8 * 16)
            io64 = c_sb[:, 0:64]
            io8 = c_sb[:, 64:72]
            for b in range(NBATCH):
                vl = b // BPV
                if b >= 2:
                    vector.wait_ge(used, b - 1)
                lhs = lhs_buf[b % 2][:]
                rhs = rhs_buf[b % 2][:]
                t0 = (b % (NT // B)) * B
                fld = {nm: f_sb[nm][:, t0:t0 + B] for nm in FIELD_NAMES}

                d, dx = scr[0][:], scr[1][:]
                p, q = scr[2][:], scr[3][:]
                px, qx = scr[4][:], scr[5][:]
                fm1, fm2 = scr_s[0][:], scr_s[1][:]
                # G1: independent ops
                vector.tensor_tensor(out=d, in0=repo(io64, B, 64),
                                     in1=repi(fld["ylo"], 64), op=AluOp.subtract)
                vector.tensor_tensor(out=dx, in0=repo(io64, B, 64),
                                     in1=repi(fld["xlo"], 64), op=AluOp.subtract)
                vector.tensor_scalar_add(fm1, fld["fyp1"], -2.0)
                vector.tensor_scalar_add(fm2, fld["fxp1"], -2.0)
                vector.tensor_tensor(out=a8_sb[:], in0=repo(io8, B, 8),
                                     in1=repi(fld["yhi"], 8), op=AluOp.is_equal)
                vector.drain()
                # G2: reads d, dx, fm1, fm2
                vector.tensor_tensor(out=p, in0=repi(fld["fyp1"], 64), in1=d,
                                     op=AluOp.subtract)
                vector.tensor_tensor(out=q, in0=d, in1=repi(fm1, 64),
                                     op=AluOp.subtract)
                vector.tensor_tensor(out=px, in0=repi(fld["fxp1"], 64), in1=dx,
                                     op=AluOp.subtract)
                vector.tensor_tensor(out=qx, in0=dx, in1=repi(fm2, 64),
                                     op=AluOp.subtract)
                vector.drain()
                # G3: mins (reuse d, dx)
                hm, hmx = d, dx
                vector.tensor_tensor(out=hm, in0=p, in1=q, op=AluOp.min)
                vector.tensor_tensor(out=hmx, in0=px, in1=qx, op=AluOp.min)
                vector.drain()
                # G4: relu (reuse p, px)
                hy, hx = p, px
                vector.tensor_scalar_max(hy, hm, 0.0)
                vector.tensor_scalar_max(hx, hmx, 0.0)
                vector.drain()
                # G5: operand outputs
                vector.tensor_tensor(out=lhs[:, :, 0:64], in0=hy,
                                     in1=repi(fld["w0"], 64), op=AluOp.mult)
                vector.tensor_tensor(out=lhs[:, :, 64:128], in0=hy,
                                     in1=repi(fld["w1"], 64), op=AluOp.mult)
                rhs4 = rhs.rearrange("p b (a x) -> p b a x", a=8)
                mm_in0 = a8_sb[:].unsqueeze(-1).broadcast_to((128, B, 8, 64))
                mm_in1 = hx.unsqueeze(2).broadcast_to((128, B, 8, 64))
                vector.tensor_tensor(out=rhs4, in0=mm_in0, in1=mm_in1,
                                     op=AluOp.mult)
                vector.drain().then_inc(bld, 1)

                # --- evictions at vl boundaries (after builds of this batch)
                if b % BPV == BPV - 1:
                    vector.wait_ge(used, (vl + 1) * BPV)
                    for xblk in range(8):
                        for dv in range(2):
                            ch = (vl % 9) - 1 + dv
                            if ch < 0 or ch >= 8:
                                continue
                            pair, sub = divmod(ch, 2)
                            src = psum[xblk][dv * 64:(dv + 1) * 64, :]\
                                .rearrange("p (a x) -> p a x", a=8)
                            dst = slab[sub * 64:(sub + 1) * 64, pair, :,
                                       xblk * 64:(xblk + 1) * 64]
                            if dv == 1:
                                vector.tensor_copy(out=dst, in_=src)
                            else:
                                vector.tensor_add(out=dst, in0=dst, in1=src)
                    vector.drain().then_inc(ev, 1)

        @block.tensor
        def _(tensor):
            for b in range(NBATCH):
                vl = b // BPV
                if b % BPV == 0 and vl > 0:
                    tensor.wait_ge(ev, vl)
                tensor.wait_ge(bld, b + 1)
                lhs = lhs_buf[b % 2]
                rhs = rhs_buf[b % 2]
                last = None
                for j in range(B):
                    t = (b % (NT // B)) * B + j
                    grp = t // CAP           # global group id = vl*8 + xblk
                    xblk = grp % 8
                    tin = t % CAP
                    last = tensor.matmul(
                        psum[xblk][:],
                        lhs[:, j],
                        rhs[:, j],
                        start=(tin == 0),
                        stop=(tin == CAP - 1),
                    )
                last.then_inc(used, 1)

    return nc


def _scatter_on_device(em_maps):
    """Run the scatter NEFF; returns full cube (64, 512, 512) f32."""
    global _scatter_nc
    import ml_dtypes
    bf16 = ml_dtypes.bfloat16

    co = np.zeros((128, 72), np.float32)
    co[:, 0:64] = np.arange(64)[None, :]
    co[:, 64:72] = np.arange(8)[None, :]
    co = co.astype(bf16)
    in_maps = [{**em_maps[c], "consts": co} for c in range(N_CORES)]

    if _scatter_nc is None:
        _scatter_nc = _build_scatter_nc()

    n_timed = int(os.environ.get("KERNEL_TIMED_RUNS", "3"))
    results, best_ns = _run_spmd_timed("scatter", _scatter_nc, in_maps, N_CORES,
                                       n_timed)
    cube = np.empty((NV, N_PIX, N_PIX), dtype=np.float32)
    for c in range(N_CORES):
        sl = results[c]["out_slab"].astype(np.float32)
        sl = sl.reshape(2, 64, 4, 8, N_PIX)     # [sub, ylo, pair, yhi, x]
        # ch = pair*2 + sub ; y = yhi*64 + ylo
        cube[c * CH_PER_CORE:(c + 1) * CH_PER_CORE] = (
            sl.transpose(2, 0, 3, 1, 4)         # [pair, sub, yhi, ylo, x]
              .reshape(CH_PER_CORE, N_PIX, N_PIX))
    return cube, best_ns


def kernel(pos_img, vel_chan, flux, kernel2d):
    global _last_exec_time_ns
    scat_ns = None
    cube = None
    if os.environ.get("KERNEL_HOST_CUBE", "0") != "1":
        try:
            em = _prep_emissions(pos_img, vel_chan, flux)
            if em is not None:
                cube, scat_ns = _scatter_on_device(em)
        except Exception as e:
            import traceback
            traceback.print_exc()
            cube = None
    if cube is None:
        cube = _host_cube(pos_img, vel_chan, flux)
    out = _conv_on_device(cube, kernel2d)
    if scat_ns is not None and _last_exec_time_ns is not None:
        _last_exec_time_ns = _last_exec_time_ns + scat_ns
    return out
